# revision 94
# baseline (speedup 1.0000x reference)
"""GQA kernel for trn2, 8 NeuronCores — bf16, phase-interleaved.

Sharding: core c = (b, g2) with b = c//4, g2 = c%4.  Each core handles batch b
and kv heads {2*g2, 2*g2+1} (q heads 8*g2 .. 8*g2+7).  Wq/Wk/Wv column-sharded
(packed as one wqkv [D,768] tensor), Wo row-sharded; host sums the 4 partial
y outputs per batch (y is emitted bf16, upcast on host).

Everything the PE touches is bf16; psum accumulation is f32.

RoPE without any rotate-half data movement for q: score(i,j) =
rope(q)·rope(k) = [q*cos; q*sin_sw] · [k_rope; swap(k_rope)] where sin_sw is
the 32-block-swapped sign-folded sin (host-precomputed) and swap is the
32-block partition swap.  So each q head stores a 128-row tile
[q*cos; q*sin_sw] (4 DVE muls, no add, no DMA), k materializes k_rope plus
its swap via partition-offset DVE copies, and the score matmul contracts
K=128 at identical PE cost (cost model charges the moving free dim only).

Program order interleaves phases so the PE never drains: warmup matmuls on a
memset tile cover the initial weight-DMA latency and pre-ramp the PE p-state,
then A0 (d-major, borrowed psum); B0 threads into A1's chain steps; B1 into
A2; B2 into A3 + C0 chunks; B3 takes C chunks as exp-latency fillers; C tail.
Inside a band the score/exp/mask stream runs one j-tile ahead of the pv
consumer (two ahead across t boundaries) and is emitted under
tc.high_priority so the scheduler sorts it in front of drain work — the pv
matmuls then never sit in the exp-latency shadow.  t-boundary filler slots
count only PE-bearing steps (A matmuls or C chunks), since epilogue-only
steps would just pile onto the DVE queue the drain itself needs.

Per-head exps fused into one joint ACT call on an adjacent psum pair; the
causal mask is one DVE mul against a host-duplicated [128,2,128] triangle.
The pv matmul's extra ones-row gives the softmax denominator; the pv-psum
handback splits its four drain ops across DVE and ACT; partition-broadcast
of the reciprocal row goes via a DRAM round-trip on the idle Pool queue for
every (band,t) except the last, which uses a PE outer-product from the pv
ring slot so the tail C chunks start immediately.  V is PE-transposed into a
bitcast bf16 psum slice.  y rows are staged in SBUF and stored with one
[128,2048] DMA per row block, per-db for the last two (HWDGE charges a
fixed ~640ns per DMA instruction across all queues, so DMA count — not
bytes — is the scarce resource; x is loaded as a handful of multi-block
column-band DMAs via DRAM-side rearrange for the same reason).
"""

import os
import numpy as np
import ml_dtypes

import concourse.bass as bass
import concourse.bacc as bacc
import concourse.mybir as mybir
import concourse.tile as tile
from concourse.bass_utils import run_bass_kernel_spmd

F32 = mybir.dt.float32
BF16 = mybir.dt.bfloat16

B, S, D = 2, 2048, 2048
H, KV, HD = 32, 8, 64
N_CORES = 8
SB = 512
NSB = S // SB     # 4
NDT = D // 128    # 16
NET = 4           # q e-tiles per core
PERM = [0, 4, 1, 5, 2, 6, 3, 7]
EXP = mybir.ActivationFunctionType.Exp

LAST_RESULT = None


def build_nc():
    nc = bacc.Bacc("TRN2", target_bir_lowering=False, debug=False,
                   enable_asserts=True, num_devices=N_CORES)

    xT = nc.dram_tensor("xT", [D, S], BF16, kind="ExternalInput")
    wqkv = nc.dram_tensor("wqkv", [D, 768], BF16, kind="ExternalInput")
    wo = nc.dram_tensor("wo", [512, D], BF16, kind="ExternalInput")
    csp = nc.dram_tensor("csp", [128, 2 * S], BF16, kind="ExternalInput")
    cmid = nc.dram_tensor("cmid", [128, SB + 128 + 256], BF16, kind="ExternalInput")
    y = nc.dram_tensor("y", [S, D], BF16, kind="ExternalOutput")
    rscratch = nc.dram_tensor("rscratch", [NSB, NET, 2, SB], BF16)  # internal

    with tile.TileContext(nc) as tc:
        with (
            tc.tile_pool(name="persist", bufs=1) as persist,
            tc.tile_pool(name="wpool", bufs=1) as wpool,
            tc.tile_pool(name="xpool", bufs=1) as xpool,
            tc.tile_pool(name="apool", bufs=2, space="PSUM") as apool,
            tc.tile_pool(name="scpool", bufs=2, space="PSUM") as scpool,
            tc.tile_pool(name="pvpool", bufs=1, space="PSUM") as pvpool,
            tc.tile_pool(name="atmp", bufs=3) as atmp,
            tc.tile_pool(name="epool", bufs=5) as epool,
            tc.tile_pool(name="btmp", bufs=2) as btmp,
            tc.tile_pool(name="ypool", bufs=2) as ypool,
        ):
            # ---- persistent SBUF ----
            # qT[t][sb]: [:, h, :] = [q_h*cos ; q_h*sin_sw]  (128 = 2x64 rows)
            qT_sb = [[persist.tile([128, 2, SB], BF16, name=f"qT{t}_{s_}")
                      for s_ in range(NSB)] for t in range(NET)]
            # kT{0,1}[sb]: [k_rope_h ; swap32(k_rope_h)]
            kT0_sb = [persist.tile([128, SB], BF16, name=f"kT0_{s_}")
                      for s_ in range(NSB)]
            kT1_sb = [persist.tile([128, SB], BF16, name=f"kT1_{s_}")
                      for s_ in range(NSB)]
            v_ones0 = [persist.tile([128, 4, 65], BF16, name=f"v_ones0_{s_}")
                       for s_ in range(NSB)]
            v_ones1 = [persist.tile([128, 4, 65], BF16, name=f"v_ones1_{s_}")
                       for s_ in range(NSB)]
            outT = [[persist.tile([128, SB], BF16, name=f"outT{t}_{s_}")
                     for s_ in range(NSB)] for t in range(NET)]
            csp_sb = persist.tile([128, 2 * S], BF16, name="csp_sb")
            cos_sb = csp_sb[:, 0:S]
            sinp_sb = csp_sb[:, S:2 * S]
            cmid_sb = persist.tile([128, SB + 128 + 256], BF16, name="cmid_sb")
            cmask_sb = cmid_sb[:, 0:SB]
            ident_sb = cmid_sb[:, SB:SB + 128]
            tri2 = cmid_sb[:, SB + 128:SB + 384].rearrange(
                "p (h c) -> p h c", h=2, c=128)
            ones_col = persist.tile([128, 4, 1], BF16, name="ones_col")
            ones_sb = persist.tile([128, 64], BF16, name="ones_sb")
            # warmup source: borrow outT[0][0] (first written at B0's drain,
            # long after the warmup matmuls' last read)
            warm_sb = outT[0][0]
            nc.vector.memset(ones_sb[:], 1.0)
            nc.gpsimd.memset(warm_sb[:], 0.0)
            nc.gpsimd.memset(ones_col[:], 1.0)
            for s_ in range(NSB):
                nc.vector.tensor_copy(v_ones0[s_][:, :, 64:65], ones_col[:])
                nc.vector.tensor_copy(v_ones1[s_][:, :, 64:65], ones_col[:])

            wqkv_sb = [wpool.tile([128, 768], BF16, name=f"wqkv{d}") for d in range(NDT)]
            wo_sb = [wpool.tile([128, D], BF16, name=f"wo{f}") for f in range(4)]
            xs = xpool.tile([128, NDT, S], BF16, name="xs")

            # ---- warmup: keep the PE busy + p-state ramped while the first
            # weight/x DMAs land.  Reads the memset tile, writes an unused
            # psum slot.
            warmp = apool.tile([128, SB], F32, name="warm", tag="acc")
            for _ in range(4):
                nc.tensor.matmul(warmp[0:64, 0:64], ones_sb[:], ones_sb[:],
                                 start=True, stop=True)
            for _ in range(6):
                nc.tensor.matmul(warmp[:], warm_sb[:, 0:128], warm_sb[:],
                                 start=True, stop=True)

            # ---- prefetch DMAs ----
            # HWDGE charges a fixed ~630ns per DMA instruction across ALL
            # queues, so x is fetched column-band by column-band with ONE
            # multi-block DMA per (d-range, 512-col band) instead of per-d
            # transfers: src partition blocks come from a DRAM rearrange.
            def x_band(dlo, dhi, clo, chi):
                src = xT[dlo * 128:dhi * 128, clo:chi].rearrange(
                    "(d p) c -> p d c", d=dhi - dlo, p=128)
                nc.sync.dma_start(xs[:, dlo:dhi, clo:chi], src)

            nc.scalar.dma_start(wqkv_sb[0][:], wqkv[0:128, :])
            nc.scalar.dma_start(wqkv_sb[1][:], wqkv[128:256, :])
            x_band(0, 4, 0, 512)
            for d in range(2, 6):
                nc.scalar.dma_start(wqkv_sb[d][:], wqkv[d * 128:(d + 1) * 128, :])
            x_band(4, 8, 0, 512)
            for d in range(6, 10):
                nc.scalar.dma_start(wqkv_sb[d][:], wqkv[d * 128:(d + 1) * 128, :])
            x_band(8, 12, 0, 512)
            for d in range(10, NDT):
                nc.scalar.dma_start(wqkv_sb[d][:], wqkv[d * 128:(d + 1) * 128, :])
            x_band(12, 16, 0, 512)
            # consts: sb0 cos/sin slices first (A0's epilogues), then the rest
            nc.scalar.dma_start(csp_sb[:, 0:SB], csp[:, 0:SB])
            nc.scalar.dma_start(csp_sb[:, S:S + SB], csp[:, S:S + SB])
            nc.scalar.dma_start(cmid_sb[:], cmid[:])
            nc.scalar.dma_start(csp_sb[:, SB:S], csp[:, SB:S])
            nc.scalar.dma_start(csp_sb[:, S + SB:2 * S], csp[:, S + SB:2 * S])
            # A1's x band, then A2/A3's
            x_band(0, 8, 512, 1024)
            x_band(8, 16, 512, 1024)
            x_band(0, 16, 1024, 1536)
            x_band(0, 16, 1536, 2048)

            tri = cmask_sb[:, 0:128]

            # ---------------- phase A ----------------
            # RoPE epilogues (see module docstring):
            #  q chain t: qT[:,h,:] <- [qtmp_h*cos ; qtmp_h*sin_sw]
            #  k chain:  kt_h[0:64] = t1_h + swap32(t2'_h);
            #            kt_h[64:128] = swap32(kt_h[0:64])
            SW = ((0, 32), (32, 0), (64, 96), (96, 64))

            def copy_from(acc, dst, eng):
                if eng == "act":
                    nc.scalar.copy(dst[:], acc[:])
                elif eng == "pool":
                    nc.gpsimd.tensor_copy(dst[:], acc[:])
                else:
                    nc.vector.tensor_copy(dst[:], acc[:])

            def rope_q(t, sb, acc, eng):
                # generator: yields between DVE ops so a concurrent B phase
                # can slip its mask/drain DVE work into the in-order queue
                scol = slice(sb * SB, (sb + 1) * SB)
                qtmp = atmp.tile([128, SB], BF16, name="qtmp", tag="qtmp",
                                 bufs=4)
                copy_from(acc, qtmp, eng)
                yield
                dst = qT_sb[t][sb]
                for h in range(2):
                    hs = slice(64 * h, 64 * h + 64)
                    with tc.high_priority(offset=50):
                        nc.vector.tensor_mul(dst[0:64, h, :], qtmp[hs, :],
                                             cos_sb[hs, scol])
                        nc.vector.tensor_mul(dst[64:128, h, :], qtmp[hs, :],
                                             sinp_sb[hs, scol])
                    yield

            def rope_k(sb, acc, eng):
                scol = slice(sb * SB, (sb + 1) * SB)
                ktmp = atmp.tile([128, SB], BF16, name="qtmp", tag="qtmp",
                                 bufs=4)
                copy_from(acc, ktmp, eng)
                yield
                t1k = atmp.tile([128, SB], BF16, name="t1k", tag="t1k", bufs=1)
                t2k = atmp.tile([128, SB], BF16, name="t2k", tag="t2k", bufs=1)
                krot = atmp.tile([128, SB], BF16, name="krot", tag="krot", bufs=1)
                nc.vector.tensor_mul(t1k[:], ktmp[:], cos_sb[:, scol])
                nc.vector.tensor_mul(t2k[:], ktmp[:], sinp_sb[:, scol])
                yield
                for (a, b_) in SW:
                    nc.vector.tensor_copy(krot[b_:b_ + 32, :], t2k[a:a + 32, :])
                yield
                kt0, kt1 = kT0_sb[sb], kT1_sb[sb]
                nc.vector.tensor_add(kt0[0:64, :], t1k[0:64, :], krot[0:64, :])
                nc.vector.tensor_add(kt1[0:64, :], t1k[64:128, :],
                                     krot[64:128, :])
                yield
                for kt in (kt0, kt1):
                    for (a, b_) in SW[:2]:
                        nc.vector.tensor_copy(kt[64 + b_:64 + b_ + 32, :],
                                              kt[a:a + 32, :])

            def v_epilogue(sb, acc, eng, trts):
                vtmp = atmp.tile([128, SB], BF16, name="vtmp", tag="vtmp",
                                 bufs=2)
                copy_from(acc, vtmp, eng)
                yield
                for u in range(4):
                    usl = slice(u * 128, (u + 1) * 128)
                    if trts is None:
                        trt = apool.tile([128, SB], F32, name="tr",
                                         tag="acc")[:]
                    else:
                        ak = trts[u]
                        trt = ak[:] if hasattr(ak, "tensor") else ak
                    tr = trt[:, 0:64].bitcast(BF16)
                    nc.tensor.transpose(tr[:], vtmp[:, usl], ident_sb[:])
                    nc.vector.tensor_copy(v_ones0[sb][:, u, 0:64], tr[:, 0:64])
                    nc.vector.tensor_copy(v_ones1[sb][:, u, 0:64], tr[:, 64:128])
                    if u % 2 == 1:
                        yield

            # chains: k first (B needs it earliest), then q0..q3, then v
            CHAINS = [("k", slice(512, 640))] + \
                     [(t, slice(t * 128, (t + 1) * 128)) for t in range(NET)] + \
                     [("v", slice(640, 768))]

            def chain_epilogue(o, sb, acc, eng, trts=None):
                if o == "k":
                    return rope_k(sb, acc, eng)
                elif o == "v":
                    return v_epilogue(sb, acc, eng, trts)
                else:
                    return rope_q(o, sb, acc, eng)

            def run_A0():
                # d-major: x tiles arrive slower than one chain consumes
                # them, so run all 6 chains per tile.  B psum slots are idle
                # here — borrow them.
                xsl = lambda d: xs[:, d, 0:512]
                scjt = scpool.tile([128, 2, SB], F32, name="scja", tag="scj")
                accs = [apool.tile([128, SB], F32, name="acc", tag="acc"),
                        apool.tile([128, SB], F32, name="acc", tag="acc"),
                        pvpool.tile([128, SB], F32, name="pva", tag="pvA"),
                        pvpool.tile([128, SB], F32, name="pvb", tag="pvB"),
                        scjt[:, 0, :], scjt[:, 1, :]]
                accof = {o: accs[ci] for ci, (o, wcol) in enumerate(CHAINS)}
                for d in range(NDT):
                    st, sp = d == 0, d == NDT - 1
                    for ci, (o, wcol) in enumerate(CHAINS):
                        ac = accof[o]
                        av = ac[:] if hasattr(ac, "tensor") else ac
                        nc.tensor.matmul(av, wqkv_sb[d][:, wcol], xsl(d),
                                         start=st, stop=sp)
                for ci, (o, wcol) in enumerate(CHAINS):
                    ac = accof[o]
                    acc = ac if not hasattr(ac, "tensor") else ac
                    if o == "v":
                        # borrow B-phase slots so A1's chains don't queue
                        # behind the transpose drains on the acc ring
                        scjt2 = scpool.tile([128, 2, SB], F32, name="scjb",
                                            tag="scj")
                        trts = [pvpool.tile([128, SB], F32, name="pvat",
                                            tag="pvA"),
                                pvpool.tile([128, SB], F32, name="pvbt",
                                            tag="pvB"),
                                scjt2[:, 0, :], scjt2[:, 1, :]]
                        for _ in chain_epilogue(o, 0, acc, "act", trts):
                            pass
                    else:
                        for _ in chain_epilogue(o, 0, acc, "act"):
                            pass

            def gen_A(sb):
                # chain-major generator: yields after every couple of
                # matmuls so a concurrent B phase can thread these steps
                # into its exp-latency slots
                xoff = sb * SB
                xsl = lambda d: xs[:, d, xoff:xoff + 512]
                for ci, (o, wcol) in enumerate(CHAINS):
                    acc = apool.tile([128, SB], F32, name="acc", tag="acc")
                    for d0 in range(0, NDT, 2):
                        for d in (d0, d0 + 1):
                            nc.tensor.matmul(acc[:], wqkv_sb[d][:, wcol],
                                             xsl(d), start=(d == 0),
                                             stop=(d == NDT - 1))
                        yield True   # PE work
                    # psum->bf16 copies go to ACT (DVE is near-saturated
                    # in the B windows; Pool cannot read PSUM)
                    for _ in chain_epilogue(o, sb, acc, "act"):
                        yield False  # DVE/ACT-only step
                    yield False

            # ---------------- phase C chunks ----------------
            c_state = {"ys": None}

            def emit_C_chunk(stt, db):
                srow = slice(stt * 128, (stt + 1) * 128)
                dcol = slice(db * SB, (db + 1) * SB)
                if c_state.get("tail"):
                    # after B3 the whole psum is free: rotate chunk psums
                    # across tags so the store-from-psum DMA latency never
                    # blocks the next chunk's matmuls
                    rot = c_state.get("rot", 0)
                    c_state["rot"] = rot + 1
                    kind = rot % 4
                    if kind in (0, 1):
                        yp = apool.tile([128, SB], F32, name="yp", tag="acc")[:]
                    elif kind == 2:
                        yp = pvpool.tile([128, SB], F32, name="ypA", tag="pvA")[:]
                    else:
                        yp = pvpool.tile([128, SB], F32, name="ypB", tag="pvB")[:]
                else:
                    yp = apool.tile([128, SB], F32, name="yp", tag="acc")[:]
                ssl = slice((stt % 4) * 128, (stt % 4 + 1) * 128)
                for f in range(4):
                    nc.tensor.matmul(yp, outT[f][stt // 4][:, ssl],
                                     wo_sb[f][:, dcol],
                                     start=(f == 0), stop=(f == 3))
                if db == 0:
                    c_state["ys"] = ypool.tile([128, D], BF16, name="ys", tag="ys")
                ys = c_state["ys"]
                if c_state.get("tail"):
                    nc.scalar.copy(ys[:, dcol], yp)   # ACT is idle in the tail
                else:
                    nc.vector.tensor_copy(ys[:, dcol], yp)
                if stt >= 14:
                    # final row blocks: store per-db so the last DMA is
                    # small and starts as early as possible
                    nc.sync.dma_start(y[srow, dcol], ys[:, dcol])
                elif db == 3:
                    nc.sync.dma_start(y[srow, :], ys[:])

            c_chunks = []          # ready-to-emit (stt, db) list, FIFO

            # ---------------- phase B ----------------
            def emit_B(bi, fillers=(), every=4, hold_c=0):
                if bi == 0:
                    for f_ in range(4):
                        nc.sync.dma_start(wo_sb[f_][:],
                                          wo[f_ * 128:(f_ + 1) * 128, :])
                njt = 4 * bi + 4

                def sc_exp(t, jt):
                    # scores + joint exp + causal mask for one j-tile;
                    # returns what pv needs later.  Emitted one tile AHEAD
                    # of the pv consumer so the in-order PE never sits in
                    # the exp latency shadow.
                    js, ju = jt // 4, jt % 4
                    jcol = slice(ju * 128, (ju + 1) * 128)
                    ro = jt - 4 * bi
                    lo = 128 * max(ro, 0)
                    qt = qT_sb[t][bi]
                    scj = scpool.tile([128, 2, SB], F32, name="scj", tag="scj")
                    import contextlib
                    hp = tc.high_priority(offset=100)
                    with hp:
                        nc.tensor.matmul(scj[:, 0, lo:], kT0_sb[js][:, jcol],
                                         qt[:, 0, lo:], start=True, stop=True)
                        nc.tensor.matmul(scj[:, 1, lo:], kT1_sb[js][:, jcol],
                                         qt[:, 1, lo:], start=True, stop=True)
                        ej = epool.tile([128, 2, SB], BF16, name="ej", tag="ej")
                        nc.scalar.activation(ej[:, :, lo:], scj[:, :, lo:],
                                             EXP, scale=0.125)
                    eA, eB = ej[:, 0, :], ej[:, 1, :]
                    if ro >= 0:
                        # the mask mul gates pv: sort it (and the whole
                        # score/exp stream) ahead of drains and fillers
                        with tc.high_priority(offset=100):
                            nc.vector.tensor_mul(ej[:, :, lo:lo + 128],
                                                 ej[:, :, lo:lo + 128], tri2)
                    return (js, ju, lo, eA, eB)

                for t in range(NET):
                    pvA = pvpool.tile([65, SB], F32, name="pvA", tag="pvA")
                    pvB = pvpool.tile([65, SB], F32, name="pvB", tag="pvB")
                    # depth-2 score pipeline: two tiles of genuine PE work
                    # overlap the previous pair's drain, and the new tile's
                    # exp latency is fully hidden
                    pend = sc_exp(t, 0)
                    pend1 = sc_exp(t, 1)
                    if t > 0:
                        # cover the previous pv pair's drain latency
                        # (pvpool bufs=1) with filler work that actually
                        # feeds the PE: epilogue-only generator steps don't
                        # count (and C chunks count double)
                        steps = 0
                        pulls = 0
                        tgt = 8
                        for f in fillers:
                            while steps < tgt and pulls < 20:
                                r = f()
                                pulls += 1
                                if not r:
                                    break
                                if r == "mm":
                                    steps += 1
                                elif r is True:   # c_filler chunk
                                    steps += 2
                            if steps >= tgt:
                                break
                    for jt in range(njt):
                        js, ju, lo, eA, eB = pend
                        st, sp = jt == 0, jt == njt - 1
                        if jt % every == every - 1:
                            # filler BEFORE the pv pair: by the time the PE
                            # reaches pv, the exp semaphore has landed
                            for f in fillers:
                                if f():
                                    break
                        nxt = sc_exp(t, jt + 2) if jt + 2 < njt else None
                        nc.tensor.matmul(pvA[:, lo:], v_ones0[js][:, ju, :],
                                         eA[:, lo:], start=st, stop=sp)
                        nc.tensor.matmul(pvB[:, lo:], v_ones1[js][:, ju, :],
                                         eB[:, lo:], start=st, stop=sp)
                        pend = pend1
                        pend1 = nxt
                    # drains: the pv psum pair is handed back only after
                    # outT copies + denominator reciprocals; split the four
                    # ops across DVE (A half) and ACT (B half) so the ring
                    # turnaround halves
                    rAB = btmp.tile([128, 2, SB], BF16, name="rAB", tag="rAB",
                                    bufs=1)
                    nc.vector.tensor_copy(outT[t][bi][0:64, :], pvA[0:64, :])
                    nc.scalar.copy(outT[t][bi][64:128, :], pvB[0:64, :])
                    with nc.allow_low_precision(reason="bf16 softmax recip"):
                        nc.vector.reciprocal(rAB[64:65, 0, :], pvA[64:65, :])
                        nc.vector.reciprocal(rAB[64:65, 1, :], pvB[64:65, :])
                    if bi == 3 and t == NET - 1:
                        # reserved chunks: independent PE work covering the
                        # final recip->bcp->norm latency chain
                        for _ in range(3):
                            if c_chunks:
                                emit_C_chunk(*c_chunks.pop(0))
                    if bi < 3 or t < NET - 1:
                        # partition-broadcast via DRAM round-trip on the idle
                        # Pool queue (frees the PE outer-product matmuls);
                        # bf16 bc also gives the norm muls DVE 2x mode.
                        # Used for every (bi,t) except the very last one:
                        # only that norm gates the tail C chunks.
                        nc.gpsimd.dma_start(rscratch[bi, t], rAB[64:65, :, :])
                        bc = btmp.tile([128, SB], BF16, name="bc", tag="bc",
                                       bufs=2)
                        nc.gpsimd.dma_start(
                            bc[0:64, :],
                            rscratch[bi, t, 0:1, :].broadcast_to((64, SB)))
                        nc.gpsimd.dma_start(
                            bc[64:128, :],
                            rscratch[bi, t, 1:2, :].broadcast_to((64, SB)))
                        nc.vector.tensor_mul(outT[t][bi][:],
                                             outT[t][bi][:], bc[:])
                    else:
                        # the last norm feeds the tail C chunks: use the
                        # low-latency PE outer-product broadcast instead of
                        # the DMA round-trip so the tail doesn't stall.  The
                        # psum comes from the pv ring slot (its natural
                        # predecessor reads — outT copy + recip — are
                        # exactly bcp's dependencies), not the acc ring,
                        # which would chain it behind C-chunk drains.
                        bcp = pvpool.tile([128, SB], F32, name="bcp", tag="pvA")
                        nc.tensor.matmul(bcp[0:64, :], ones_sb[64:65, :],
                                         rAB[64:65, 0, :], start=True, stop=True)
                        nc.tensor.matmul(bcp[64:128, :], ones_sb[64:65, :],
                                         rAB[64:65, 1, :], start=True, stop=True)
                        nc.vector.tensor_mul(outT[t][bi][:],
                                             outT[t][bi][:], bcp[:])
                # this bi's output rows are ready for phase C
                for stt in range(4 * bi, 4 * bi + 4):
                    for db in range(4):
                        c_chunks.append((stt, db))

            # ---------------- program order ----------------
            gA = {"g": None}

            def a_filler():
                # returns "mm" for a PE-matmul step, "ep" for an
                # epilogue-only step, False when exhausted
                if gA["g"] is None:
                    return False
                try:
                    tag = next(gA["g"])
                    return "mm" if tag else "ep"
                except StopIteration:
                    gA["g"] = None
                    return False

            c_floor = {"n": 0}

            def c_filler():
                if len(c_chunks) > c_floor["n"]:
                    emit_C_chunk(*c_chunks.pop(0))
                    return True
                return False

            run_A0()
            gA["g"] = gen_A(1)
            emit_B(0, fillers=[lambda: bool(a_filler()) | bool(a_filler())],
                   every=1)
            while a_filler():
                pass
            gA["g"] = gen_A(2)
            emit_B(1, fillers=[a_filler, c_filler], every=1)
            while a_filler():
                pass
            gA["g"] = gen_A(3)
            emit_B(2, fillers=[a_filler, c_filler], every=1)
            while a_filler():
                pass
            c_floor["n"] = 3   # hold chunks back for the final-norm cover
            emit_B(3, fillers=[c_filler], every=3)
            c_state["tail"] = True
            while c_chunks:
                emit_C_chunk(*c_chunks.pop(0))

    nc.compile()
    return nc


def host_inputs(x, Wq, Wk, Wv, Wo):
    """Per-core input maps (8 cores)."""
    BF = ml_dtypes.bfloat16
    inv = 1.0 / (10000.0 ** (np.arange(0, HD, 2, dtype=np.float64) / HD))
    freqs = np.outer(np.arange(S, dtype=np.float64), inv)          # [S, 32]
    emb = np.concatenate([freqs, freqs], axis=1)                   # [S, 64]
    cos = np.cos(emb).astype(np.float32)
    sin = np.sin(emb).astype(np.float32)
    cos2 = np.ascontiguousarray(np.tile(cos.T, (2, 1)))            # [128, S]
    sinf = np.concatenate([-sin[:, :32], sin[:, 32:]], axis=1)     # sign-folded
    sin2 = np.ascontiguousarray(np.tile(sinf.T, (2, 1)))           # [128, S]
    # 32-block swap within each 64-row half (rows 64:128 repeat 0:64)
    sinp2 = np.concatenate([sin2[32:64], sin2[0:32],
                            sin2[96:128], sin2[64:96]], axis=0)
    csp = np.ascontiguousarray(
        np.concatenate([cos2, sinp2], axis=1)).astype(BF)          # [128, 2S]
    j = np.arange(128)[:, None]
    i = np.arange(SB)[None, :]
    cmask = (j <= i).astype(BF)                                    # [128, 512]
    ident = np.eye(128, dtype=BF)
    tri = cmask[:, 0:128]
    cmid = np.ascontiguousarray(
        np.concatenate([cmask, ident, tri, tri], axis=1)).astype(BF)  # [128, 896]

    Wq4 = Wq.reshape(D, H, HD)
    Wo4 = Wo.reshape(H, HD, D)
    Wk4 = Wk.reshape(D, KV, HD)
    Wv4 = Wv.reshape(D, KV, HD)

    maps = []
    for c in range(N_CORES):
        b, g2 = c // 4, c % 4
        gh = [8 * g2 + p for p in PERM]
        wq_c = Wq4[:, gh, :].reshape(D, 512)
        wk_c = Wk4[:, [2 * g2, 2 * g2 + 1], :].reshape(D, 128)
        wv_c = Wv4[:, [2 * g2, 2 * g2 + 1], :].reshape(D, 128)
        maps.append({
            "xT": np.ascontiguousarray(x[b].T).astype(BF),
            "wqkv": np.ascontiguousarray(
                np.concatenate([wq_c, wk_c, wv_c], axis=1)).astype(BF),
            "wo": np.ascontiguousarray(Wo4[gh].reshape(512, D)).astype(BF),
            "csp": csp, "cmid": cmid,
        })
    return maps


_NC_CACHE = None


def kernel(x, Wq, Wk, Wv, Wo):
    global LAST_RESULT, _NC_CACHE
    x = np.asarray(x, np.float32)
    maps = host_inputs(np.asarray(x, np.float32), np.asarray(Wq, np.float32),
                       np.asarray(Wk, np.float32), np.asarray(Wv, np.float32),
                       np.asarray(Wo, np.float32))
    if _NC_CACHE is None:
        _NC_CACHE = build_nc()
    trace = bool(os.environ.get("KERNEL_TRACE"))
    try:
        res = run_bass_kernel_spmd(_NC_CACHE, maps, list(range(N_CORES)), trace=trace)
    except (ImportError, ModuleNotFoundError):
        res = run_bass_kernel_spmd(_NC_CACHE, maps, list(range(N_CORES)), trace=False)
    LAST_RESULT = res
    out = np.zeros((B, S, D), np.float32)
    for b in range(B):
        for g2 in range(4):
            out[b] += np.asarray(res.results[4 * b + g2]["y"], np.float32)
    return out
